# revision 35
# baseline (speedup 1.0000x reference)
"""Trainium2 Bass kernel for StyleGAN2-style modulated conv2d (ModConv2D).

Reference computation (per sample b):
    w      = kernel * (style[b] + 1)                 # modulate [3,3,Cin,Cout]
    w      = w / sqrt(sum(w^2, (kh,kw,Cin)) + 1e-8)  # demodulate per Cout
    y[b]   = conv2d_same(x[b], w)

Sharding: data-parallel over batch — 16 samples across 8 NeuronCores,
2 samples per core; the base kernel is replicated.  As part of the
shard/unshard step the host hands the device x in channel-major layout
[B, Cin, H*W] and receives y channel-major [B, Cout, H*W] (the reference
itself runs the conv in NCHW); the device kernel therefore needs no
layout transposes at all — the PE does nothing but the conv.

Device algorithm per core (2 samples):
  - conv as 9-tap accumulated matmuls: psum[cout,pix] += w[t,cin,cout]^T @
    xT[cin, pix+off].  x is held channel-major FLAT ([cin, cc, 64+4096+80]
    fp16, loaded straight from DRAM with a casting DMA) with zero guard
    regions; horizontal (dx=+-1) taps use column-split matmuls (N=504,
    strided psum out) so row wrap never leaks.
  - weights are modulated on-chip per-tap (ACT ring) so the first conv
    group unblocks as the per-tap kernel DMAs land.
  - demod factor d[cout] = rsqrt(sum_cin s^2 * K2 + 1e-8) in fp32 on
    device (K2 = sum_t kernel^2, squared taps staged fp16) computed
    column-wise via N=1 matmuls (k2^T @ s2col), applied per-partition on
    psum eviction (oc0 -> scalar ACT, oc1 -> DVE; balances the rings)
    into an oc-major fp16 staging tile; ONE store per tile casts back to
    fp32 (SWDGE).  Every cross-engine handoff rides engine or plain-DMA
    semaphores (no DMA-transpose semaphores, which fire early).
  - Scheduling: conv tiles 0-1 run with their taps split (the taps that
    only need x blocks <=k run while block k+1's DMA lands) and their
    evictions deferred past the demod chain, so neither the K2 reduction
    (~7us on the DVE) nor the demod matmuls ever sit in front of conv
    work in PE order; sample 1's modulation is emitted upfront on queues
    that cannot block sample 0.  The final tile evicts per cout half and
    ships each half immediately, shortening the tail.
"""

import numpy as np

B, H, W, CIN, COUT, KH, KW = 16, 64, 64, 256, 256, 3, 3
NCORES = 8
BPC = B // NCORES  # samples per core
T = KH * KW  # 9 taps
HWPIX = H * W  # 4096
PAD0 = 64  # zero pixels before the image
XLEN = PAD0 + HWPIX + 80  # 4240

# tap order: dx=0 taps first so the first matmul of each psum group writes all
# 512 columns with start=True
TAP_ORDER = [1, 4, 7, 0, 3, 6, 2, 5, 8]

_CACHE = {}
LAST_EXEC_NS = None
LAST_MEAN_EXEC_NS = None


def _build_nc():
    from contextlib import ExitStack

    import concourse.bacc as bacc
    import concourse.bass as bass
    import concourse.mybir as mybir
    import concourse.tile as tile

    f32 = mybir.dt.float32
    f16 = mybir.dt.float16  # fp16: same 1 cyc/row PE rate as bf16, 4x finer mantissa
    AF = mybir.ActivationFunctionType

    nc = bacc.Bacc("TRN2", target_bir_lowering=False, debug=False)

    # channel-major fp16 x/y and fp16 kernel (host converts as part of
    # shard/unshard: the device consumed x/kernel at fp16 precision anyway,
    # so this halves HBM traffic at identical device arithmetic)
    x_d = nc.dram_tensor("x", [BPC, CIN, HWPIX], f16, kind="ExternalInput")
    s_d = nc.dram_tensor("style", [BPC, CIN], f32, kind="ExternalInput")
    k_d = nc.dram_tensor("kernel", [KH, KW, CIN, COUT], f16, kind="ExternalInput")
    y_d = nc.dram_tensor("y", [BPC, COUT, HWPIX], f16, kind="ExternalOutput")

    XB = CIN * HWPIX  # x/y sample stride (elements)
    KKW = CIN * COUT  # kernel tap stride

    def x_blk_ap(b, t8):
        # [128 cin, 2 cc, 512 pix] starting at pixel t8*512
        off = b * XB + t8 * 512
        return bass.AP(x_d, off, [[HWPIX, 128], [128 * HWPIX, 2], [1, 512]])

    def y_blk_ap(b, t8):
        # [128 cout, 2 oc, 512 pix]
        off = b * XB + t8 * 512
        return bass.AP(y_d, off, [[HWPIX, 128], [128 * HWPIX, 2], [1, 512]])

    def y_half_ap(b, t8, oc):
        off = b * XB + oc * 128 * HWPIX + t8 * 512
        return bass.AP(y_d, off, [[HWPIX, 128], [1, 512]])

    def k_tap_ap(cc, t):
        # [128 cin, 256 cout] for one tap
        return bass.AP(k_d, t * KKW + cc * 128 * COUT, [[COUT, 128], [1, COUT]])

    with tile.TileContext(nc) as tc, ExitStack() as ctx:
        singles = ctx.enter_context(tc.tile_pool(name="singles", bufs=1))
        tmp_pool = ctx.enter_context(tc.tile_pool(name="tmp", bufs=1))
        wpool = ctx.enter_context(tc.tile_pool(name="wpool", bufs=2))
        dpool = ctx.enter_context(tc.tile_pool(name="dpool", bufs=10))
        srow_pool = ctx.enter_context(tc.tile_pool(name="srow", bufs=4))
        xpool = ctx.enter_context(tc.tile_pool(name="xpool", bufs=2))
        ospool = ctx.enter_context(tc.tile_pool(name="osb", bufs=6))
        pconv = ctx.enter_context(tc.tile_pool(name="pconv", bufs=6, space="PSUM"))
        psmall = ctx.enter_context(tc.tile_pool(name="psmall", bufs=1, space="PSUM"))

        # style rows (tiny, first on the sync ring)
        srows = []
        for b in range(BPC):
            srow = srow_pool.tile([1, CIN], f32, tag=f"srow{b}")
            nc.sync.dma_start(out=srow, in_=s_d.ap()[b : b + 1, :])
            srows.append(srow)

        # per-tap kernel loads in conv tap order (the modulated weights gate
        # the conv ramp), alternating HWDGE rings.  Each ring moves only
        # ~100GB/s, so 9 taps/ring would take ~11.5us; the last-needed tap
        # pair (t=8) loads via the SWDGE queue between the early x blocks
        # instead, bringing the rings down to ~10us.
        kbase = singles.tile([128, 2, T, COUT], f16)
        for ti, t in enumerate(TAP_ORDER[:-1]):
            for cc in range(2):
                # (ti+cc)%2 so each ring gets an interleaved mix of cc0/cc1
                # taps ((ti*2+cc)%2 degenerates to cc: one ring would carry
                # every cc0 tap and serialize the ACT-ring wb chain)
                eng = nc.sync if (ti + cc) % 2 == 0 else nc.scalar
                eng.dma_start(out=kbase[:, cc, t], in_=k_tap_ap(cc, t))

        ones1 = singles.tile([1, 1], f32)
        nc.vector.memset(ones1, 1.0)
        eps_sb = singles.tile([128, 1], f32)
        nc.vector.memset(eps_sb, 1e-8)

        # xflat guard memsets first on the DVE (no input deps; they gate
        # conv tile 0's dy=-1 taps)
        xflats = []
        for b in range(BPC):
            xflat = xpool.tile([128, 2, XLEN], f16, tag="xflat", name=f"xflat{b}")
            nc.vector.memset(xflat[:, :, 0:PAD0], 0.0)
            nc.vector.memset(xflat[:, :, PAD0 + HWPIX : XLEN], 0.0)
            xflats.append(xflat)

        # x loads: straight into the conv layout, one casting DMA per
        # 512-pixel block (SWDGE; each gpsimd issue costs ~870ns, so whole
        # blocks beat finer granularity).  Only the first four blocks are
        # issued upfront — issuing all 8.4MB at once starves the kernel-tap
        # DMAs for HBM (observed: taps land ~19us instead of ~8), and the
        # taps gate the whole conv ramp via the modulated weights.  The
        # rest trickle in two per conv tile from the pipeline loop.
        def load_x_blk(b, t8):
            out = xflats[b][:, :, PAD0 + 512 * t8 : PAD0 + 512 * (t8 + 1)]
            nc.gpsimd.dma_start(out=out, in_=x_blk_ap(b, t8))

        for t8 in range(3):
            load_x_blk(0, t8)
        for cc in range(2):
            nc.gpsimd.dma_start(out=kbase[:, cc, 8], in_=k_tap_ap(cc, 8))
        for t8 in range(3, 8):
            load_x_blk(0, t8)
        for t8 in range(8):
            load_x_blk(1, t8)

        wbs, dsbs, s2cs, smods = {}, {}, {}, {}

        # one bank-shared psum tile, column slots for all tiny matmuls:
        # independent columns -> no WAR serialization between the chains
        pcol8 = psmall.tile([128, 8], f32, tag="pcol8")

        def modulation(b):
            srow1 = srow_pool.tile([1, CIN], f32, tag=f"srow1_{b}")
            nc.vector.tensor_scalar_add(srow1, srows[b], 1.0)

            smod = dpool.tile([128, 2], f32, tag=f"smod{b}")  # (style+1) col-major
            for cc in range(2):
                pcol = pcol8[:, 4 * b + cc : 4 * b + cc + 1]
                nc.tensor.matmul(
                    pcol, srow1[:, cc * 128 : (cc + 1) * 128], ones1, start=True, stop=True
                )
                nc.vector.tensor_copy(out=smod[:, cc : cc + 1], in_=pcol)
            s2c = dpool.tile([128, 2], f32, tag=f"s2c{b}")
            nc.vector.tensor_mul(s2c, smod, smod)
            s2cs[b] = s2c

            # wb[cin, cc, t, cout] = kernel * (s+1), cast fp16, per tap in
            # conv order so the first conv group unblocks early.  Split
            # cc0->ACT ring / cc1->DVE: one ring alone (~0.5us per ACT)
            # cannot keep ahead of the conv's ~0.43us-per-tap consumption.
            wbs[b] = wpool.tile([128, 2, T, COUT], f16, tag="wb", name=f"wb{b}")
            smods[b] = smod

        def emit_wb_tap(b, t):
            wb, smod = wbs[b], smods[b]
            nc.scalar.activation(
                wb[:, 0, t], kbase[:, 0, t], AF.Copy, scale=smod[:, 0:1]
            )
            nc.vector.tensor_scalar_mul(wb[:, 1, t], kbase[:, 1, t], smod[:, 1:2])

        def demod(b):
            # d[cout] = rsqrt(sum_cin s2*K2 + 1e-8) as a column, via two N=1
            # matmuls per cout half (deferred past the first conv tiles)
            s2c = s2cs[b]
            sqc = dpool.tile([128, 2], f32, tag=f"sqc{b}")
            for oc in range(2):
                pcol = pcol8[:, 4 * b + 2 + oc : 4 * b + 3 + oc]
                for cc in range(2):
                    nc.tensor.matmul(
                        pcol,
                        k2[:, cc, oc * 128 : (oc + 1) * 128],
                        s2c[:, cc : cc + 1],
                        start=(cc == 0),
                        stop=(cc == 1),
                    )
                nc.scalar.activation(sqc[:, oc : oc + 1], pcol, AF.Sqrt, bias=eps_sb)
            d_sb = dpool.tile([128, 2], f32, tag=f"d{b}")
            nc.vector.reciprocal(d_sb, sqc)
            dsbs[b] = d_sb


        def mm_taps(b, t8, oc, ps, taps, first, final):
            # accumulate a subset of taps into one cout-half psum
            wb = wbs[b]
            xflat = xflats[b]
            p0 = t8 * 512
            ps_r = ps.rearrange("p (r w) -> p r w", w=64)
            n = len(taps) * 2
            i = 0
            for t in taps:
                dy, dx = t // 3 - 1, t % 3 - 1
                base = PAD0 + p0 + 64 * dy
                for cc in range(2):
                    lhsT = wb[:, cc, t, oc * 128 : (oc + 1) * 128]
                    xf = xflat[:, cc]
                    if dx == 0:
                        rhs = xf[:, base : base + 512]
                        out_ap = ps
                    elif dx == -1:
                        rhs = xf[:, base : base + 512].rearrange(
                            "p (r w) -> p r w", w=64
                        )[:, :, 0:63]
                        out_ap = ps_r[:, :, 1:64]
                    else:  # dx == +1
                        rhs = xf[:, base + 1 : base + 513].rearrange(
                            "p (r w) -> p r w", w=64
                        )[:, :, 0:63]
                        out_ap = ps_r[:, :, 0:63]
                    nc.tensor.matmul(
                        out_ap, lhsT, rhs,
                        start=(first and i == 0),
                        stop=(final and i == n - 1),
                    )
                    i += 1

        def conv_matmuls(b, t8):
            pss = []
            for oc in range(2):
                ps = pconv.tile([128, 512], f32, tag="pconv", name=f"ps_{b}_{t8}_{oc}")
                mm_taps(b, t8, oc, ps, TAP_ORDER, True, True)
                pss.append(ps)
            return pss

        def conv_evict(b, t8, pss, ship_halves=False):
            # demod scale + fp32->fp16 into an oc-major staging tile; oc0 on
            # the ACT ring, oc1 on the DVE so neither carries both.  One
            # store per tile (engine-written staging -> sound semaphores).
            d_sb = dsbs[b]
            osb = ospool.tile([128, 2, 512], f16, tag="osb", name=f"osb_{b}_{t8}")
            nc.scalar.activation(
                osb[:, 0], pss[0], AF.Copy, scale=d_sb[:, 0:1]
            )
            nc.vector.tensor_scalar_mul(osb[:, 1], pss[1], d_sb[:, 1:2])
            nc.gpsimd.dma_start(out=y_blk_ap(b, t8), in_=osb)

        # K2[cin, cout] = sum_t kernel^2 (once per core).  Squared taps are
        # staged fp16 (2x DVE read rate on the reduce); accumulation and k2
        # stay fp32.  Emitted after the startup-critical DVE work; the demod
        # matmuls that consume it are deferred past conv tiles 0-1 so the PE
        # never waits for it.
        k2 = singles.tile([128, 2, COUT], f32)

        def compute_k2():
            for cc in range(2):
                k2tmp = tmp_pool.tile([128, T, COUT], f16, tag="k2tmp")
                nc.vector.tensor_mul(k2tmp, kbase[:, cc], kbase[:, cc])
                nc.vector.reduce_sum(
                    out=k2[:, cc],
                    in_=k2tmp.rearrange("p t c -> p c t"),
                    axis=mybir.AxisListType.X,
                )

        modulation(0)
        for t in TAP_ORDER:
            emit_wb_tap(0, t)
        compute_k2()  # on the DVE right after sample 0's wb muls

        items = [(b, t8) for b in range(BPC) for t8 in range(8)]

        # Ramp: tiles 0 and 1 run with their taps split so the PE does the
        # taps that only need x blocks <=k while block k+1's DMA lands.
        # Their evictions are deferred past demod so the demod matmuls
        # (which wait on K2) never sit in front of conv work in PE order.
        TAPS_LO = [1, 4, 0, 3, 2]  # touch nothing past pixel p0+511
        TAPS_HI = [5, 7, 6, 8]  # dx=+1 / dy=+1: read into the next block
        ps0 = [pconv.tile([128, 512], f32, tag="pconv", name=f"ps0_{oc}") for oc in range(2)]
        for oc in range(2):
            mm_taps(0, 0, oc, ps0[oc], TAPS_LO, True, False)
        for oc in range(2):
            mm_taps(0, 0, oc, ps0[oc], TAPS_HI, False, True)
        ps1 = [pconv.tile([128, 512], f32, tag="pconv", name=f"ps1_{oc}") for oc in range(2)]
        for oc in range(2):
            mm_taps(0, 1, oc, ps1[oc], TAPS_LO, True, False)
        for oc in range(2):
            mm_taps(0, 1, oc, ps1[oc], TAPS_HI, False, True)
        demod(0)
        conv_evict(*items[0], ps0)
        conv_evict(*items[1], ps1)
        # sample 1's modulation/demod staged through the early loop
        # iterations so none of it can delay sample 0's eviction gates
        # (recip0) or psum drainage; its wb taps trickle 2 per tile.
        wb1_taps = list(TAP_ORDER)
        for i, (b, t8) in enumerate(items):
            if i < 2:
                continue
            if BPC > 1:
                if i == 3:
                    modulation(1)
                elif i == 4:
                    demod(1)
                elif i >= 5 and wb1_taps:
                    emit_wb_tap(1, wb1_taps.pop(0))
                    emit_wb_tap(1, wb1_taps.pop(0))
                    if i == 5:
                        emit_wb_tap(1, wb1_taps.pop(0))
            if i == len(items) - 1:
                # last tile per-oc: oc0's evict/store overlaps oc1's
                # matmuls, shortening the tail
                d_sb = dsbs[b]
                for oc in range(2):
                    ps = pconv.tile([128, 512], f32, tag="pconv", name=f"ps_last_{oc}")
                    mm_taps(b, t8, oc, ps, TAP_ORDER, True, True)
                    o_sb = ospool.tile([128, 512], f16, tag="oslast", name=f"osl_{oc}")
                    nc.vector.tensor_scalar_mul(o_sb, ps, d_sb[:, oc : oc + 1])
                    nc.gpsimd.dma_start(out=y_half_ap(b, t8, oc), in_=o_sb)
            else:
                conv_evict(b, t8, conv_matmuls(b, t8))

    nc.compile()
    return nc


def _get_nc():
    if "nc" not in _CACHE:
        _CACHE["nc"] = _build_nc()
    return _CACHE["nc"]


def kernel(x, style, kernel, _trace=False):
    global LAST_EXEC_NS, LAST_MEAN_EXEC_NS
    from concourse.bass_utils import run_bass_kernel_spmd

    # shard + lay out for the device: x channel-major [B, Cin, H*W]
    # (the reference itself runs its conv in NCHW)
    x = np.ascontiguousarray(
        np.transpose(np.asarray(x, dtype=np.float16).reshape(B, HWPIX, CIN), (0, 2, 1))
    )
    style = np.ascontiguousarray(style, dtype=np.float32)
    kern = np.ascontiguousarray(kernel, dtype=np.float16)

    nc = _get_nc()
    in_maps = [
        {
            "x": x[i * BPC : (i + 1) * BPC],
            "style": style[i * BPC : (i + 1) * BPC],
            "kernel": kern,
        }
        for i in range(NCORES)
    ]
    res = run_bass_kernel_spmd(nc, in_maps, core_ids=list(range(NCORES)), trace=_trace)
    LAST_EXEC_NS = res.exec_time_ns
    LAST_MEAN_EXEC_NS = res.mean_exec_time_ns
    # unshard: y channel-major [B, Cout, H*W] -> [B, H, W, Cout]
    y = np.concatenate([res.results[i]["y"] for i in range(NCORES)], axis=0)
    return np.ascontiguousarray(
        np.transpose(y.astype(np.float32), (0, 2, 1)).reshape(B, H, W, COUT)
    )
